# revision 25
# baseline (speedup 1.0000x reference)
"""Trainium2 Bass kernel for EnhancedBiologicalSplatAttentionLayer.

Reference computation (B=4, S=2048, D=1024, K=64):
    v    = x @ Wv.T                                            [B,S,D]
    aff  = normalize_k(exp(-0.5*dist_sq(x, centers)/scale^2))  [B,S,K]
    st   = aff.T @ v   (per batch)                             [B,K,D]
    tok  = aff @ st                                            [B,S,D]
    out  = tok @ Wo.T                                          [B,S,D]

Algebraic reduction (exact reassociation, weights folded on host):
    Q = Wv.T @ Wo.T [D,D] (host fp32; per-core half Qj [D,512]), then
    out_j = aff @ P with   P = sum_s_blocks P_contrib(block):
      R-route (s-pairs 0,1,6,7):  P += aff_i.T @ (x_i @ Qj)   [K,512]
      M-route (s-pairs 2..5):     P += (aff_i.T @ x_i) @ Qj
The R-route contracts x with Q first, so it only ever reads x.T (already
resident for the affinity matmuls); the M-route needs an s-major copy of
its x rows (the 1MB "xn" input) but costs 6x fewer PE cycles.  The split
balances DMA (x.T 2MB + xn 1MB + Q 0.5MB in, bf16 out 2MB) against
PE/eviction load, and routes the first/last s-quads through R so the
final P contributions never wait for the xn stream.

The affinity exponent is E[s,k] = g_k*(x_s.c_k) + a_k*|x_s|^2 + b_k with
g = 1/scale^2, a = -0.5g, b = -0.5|c|^2 g folded on host into the fp8
centers operand, plus a [2, K+S] bf16 "aux" tensor carrying
(b|1 ; a|xsq) so a*|x|^2 + b enters each affinity PSUM group as one
rank-2 matmul.  Per-s rounding in xsq shifts all k equally and cancels
in the k-normalization.  exp() underflows to exactly 0.0 in fp32 for
this input distribution, faithfully matching the fp32 reference (no
softmax max-subtraction; +EPS enters the denominator as a built-in
exp(ln_eps)-style column of the au tile).

Sharding over 8 cores, no cross-core communication:
    core c -> batch b = c//2, output-dim half j = c%2; each core runs the
    affinity pipeline + R/M/P for its batch and emits
    out[b][:, j*512:(j+1)*512] (bf16, host upcasts to fp32).

Scheduling notes (tuned against the TRN2 instruction cost model):
  - fp8e4m3 operands with MatmulPerfMode.DoubleRow wherever the
    contraction spans >=2 128-tiles: 0.5 cycles/output-column.
  - Affinities in [s,k] orientation: normalization is per-partition
    (reduce + reciprocal + per-partition scalar muls on DVE/GPSIMD).
  - Loads stream in compute order (xt quarters, then xn pair-chunks);
    phase A+R processes s-quads 0..3 behind the stream; quad 3's R block
    runs BEFORE its affinity chain (it needs only xt+Q), and the M waves
    sit at the end of the PE stream so xn waits block nothing.
  - ACT has a depth-0 exec queue (strict head-of-line blocking), so
    issue order on ACT keeps exp/evictions off the P critical chain;
    cross-engine parallel evictions always use separate tiles
    (dependency tracking is tile-granular).
  - Scale bookkeeping: Q ships as Q*64 (fp8 normal range), R evicts at
    x0.25, M at x0.25, P at x1/512, and the output eviction applies x32,
    which exactly cancels: out = aff @ P_true.
"""
import numpy as np
import ml_dtypes

import concourse.bass as bass
import concourse.bacc as bacc
import concourse.tile as tile
from concourse import mybir
from concourse.masks import make_identity
from concourse.bass_utils import run_bass_kernel_spmd

B, S, D, K = 4, 2048, 1024, 64
P = 128
DT = D // P          # 8 d-tiles
ST = S // P          # 16 s-tiles
F = D // 2           # 512-wide output half per core
EPS = 1e-8

QSCALE = 64.0        # host scale on Q so fp8 entries sit in normal range
RSCALE = 0.25        # R eviction scale into fp8
PEVSCALE = 1.0 / 512.0   # P eviction scale into fp8
OSCALE = 1.0 / (QSCALE * RSCALE * PEVSCALE)   # output rescale (=32)

BF = mybir.dt.bfloat16
F32 = mybir.dt.float32
FP8 = mybir.dt.float8e4
BF_NP = ml_dtypes.bfloat16
FP8_NP = ml_dtypes.float8_e4m3
DR = mybir.MatmulPerfMode.DoubleRow

_CACHE = {}


def build_nc(phase="full"):
    nc = bacc.Bacc("TRN2", target_bir_lowering=False, debug=False)

    xt_d = nc.dram_tensor("xt", [D, S], FP8, kind="ExternalInput")
    xn_d = nc.dram_tensor("xn", [1024, D], FP8, kind="ExternalInput")
    q_d = nc.dram_tensor("q8", [D, F], FP8, kind="ExternalInput")
    cts_d = nc.dram_tensor("cts", [D, K], FP8, kind="ExternalInput")
    aux_d = nc.dram_tensor("aux", [2, K + S], BF, kind="ExternalInput")
    out_d = nc.dram_tensor("out", [S, F], BF, kind="ExternalOutput")

    with tile.TileContext(nc) as tc:
        with tc.tile_pool(name="persist", bufs=1) as persist:
            ident8 = persist.tile([P, P], FP8)
            make_identity(nc, ident8)

            # ---- persistent SBUF tensors ------------------------------
            cts_sb = persist.tile([P, DT, K], FP8)     # d = p*8 + n
            aux_sb = persist.tile([2, K + S], BF)      # (b|1 ; a|xsq)
            xt_sb = persist.tile([P, DT, S], FP8)      # d = p*8 + n
            q_sb = persist.tile([P, DT, F], FP8)       # d = t*128 + p
            A_sk = persist.tile([P, ST, K], FP8)       # s = st*128 + p
            a_ks = persist.tile([K, ST, P], FP8)       # transposed slices
            xn_sb = persist.tile([P, 8, D], FP8)       # s = 512 + t*128 + p
            r_sb = persist.tile([P, 8, F], FP8)        # R*RSCALE sts 0-3,12-15
            mt_sb = persist.tile([P, DT, K], FP8)      # (M/4).T
            p_sb = persist.tile([K, F], FP8)           # P*PEVSCALE
            # exp outputs, double-buffered, with a built-in EPS column so
            # the denominator reduce includes +EPS for free
            au_sb = persist.tile([P, 2, 4, K + 1], BF)
            nc.vector.memset(au_sb[:, 0, :, K:], EPS)
            nc.vector.memset(au_sb[:, 1, :, K:], EPS)

            # ---- loads (order = stream order on the shared DMA bus) ---
            xt_v = xt_d.ap().rearrange("(p n) s -> p n s", n=DT)
            xn_v = xn_d.ap().rearrange("(t p) d -> p t d", p=P)
            nc.sync.dma_start(out=xt_sb[:, :, 0:512], in_=xt_v[:, :, 0:512])
            nc.sync.dma_start(
                out=cts_sb[:], in_=cts_d.ap().rearrange("(p n) k -> p n k", n=DT)
            )
            nc.sync.dma_start(out=aux_sb[:], in_=aux_d.ap())
            nc.sync.dma_start(
                out=q_sb[:], in_=q_d.ap().rearrange("(t p) f -> p t f", p=P)
            )
            for c in range(1, 4):
                sl = slice(c * 512, (c + 1) * 512)
                nc.sync.dma_start(out=xt_sb[:, :, sl], in_=xt_v[:, :, sl])
            for i in range(4):              # xn chunks for s-pairs 2..5
                sl = slice(2 * i, 2 * i + 2)
                nc.sync.dma_start(out=xn_sb[:, sl, :], in_=xn_v[:, sl, :])

            if phase == "dma":
                with tc.tile_pool(name="zo", bufs=2) as zo:
                    out_v = out_d.ap().rearrange("(t p) f -> p t f", p=P)
                    for u in range(ST // 2):
                        o_sb = zo.tile([P, 2, F], BF, tag="o_sb")
                        nc.vector.memset(o_sb[:], 0.0)
                        nc.sync.dma_start(
                            out=out_v[:, 2 * u:2 * u + 2, :], in_=o_sb[:]
                        )
            else:
                _emit_main(nc, tc, persist, locals())

    nc.compile()
    return nc


def _emit_main(nc, tc, persist, env):
    ident8 = env["ident8"]
    cts_sb = env["cts_sb"]; aux_sb = env["aux_sb"]
    xt_sb = env["xt_sb"]; xn_sb = env["xn_sb"]; q_sb = env["q_sb"]
    A_sk = env["A_sk"]; a_ks = env["a_ks"]; r_sb = env["r_sb"]
    mt_sb = env["mt_sb"]; p_sb = env["p_sb"]; au_sb = env["au_sb"]
    out_d = env["out_d"]

    with (
        tc.tile_pool(name="psP", bufs=1, space="PSUM") as pP,
        tc.tile_pool(name="psM", bufs=1, space="PSUM") as pM,
    ):
        ps_P = pP.tile([K, F], F32)
        ps_M = pM.tile([P, DT, K], F32)     # M bank for s-pairs 2..5
        tr_tiles = {}

        # ---- phase A in quad order 0,1,3,2: R = x@Q for quads 0,3
        # (their P parts close early); M for pairs 2-5 accumulates in
        # the LAST iteration so xn waits block nothing critical --------
        with (
            tc.tile_pool(name="pa", bufs=1, space="PSUM") as pa,
            tc.tile_pool(name="ptr", bufs=2, space="PSUM") as ptr,
            tc.tile_pool(name="pR", bufs=3, space="PSUM") as pR,
            tc.tile_pool(name="sden", bufs=2) as sden,
        ):
            def emit_xc(q):
                st0 = 4 * q
                ps_a = pa.tile([P, 4, 128], F32, tag="pa")
                for j in range(4):
                    st = st0 + j
                    o = ps_a[:, j, 0:K]
                    ssl = slice(st * P, (st + 1) * P)
                    for h in range(4):      # d-tile pairs (DoubleRow)
                        nc.tensor.matmul(
                            o, xt_sb[:, 2 * h:2 * h + 2, ssl],
                            cts_sb[:, 2 * h:2 * h + 2, :],
                            start=(h == 0), stop=False, perf_mode=DR,
                        )
                    # rank-2 term: ones(s)b_k + xsq(s)a_k
                    nc.tensor.matmul(
                        o, aux_sb[:, K + st * P:K + (st + 1) * P],
                        aux_sb[:, 0:K], start=False, stop=True,
                    )
                return ps_a

            def emit_norm(q, n, ps_a):
                st0 = 4 * q
                slot = n % 2
                # exp over the quad (underflows to exactly 0.0 here)
                nc.scalar.activation(
                    out=au_sb[:, slot, :, 0:K], in_=ps_a[:, :, 0:K],
                    func=mybir.ActivationFunctionType.Exp,
                )
                den = sden.tile([P, 4, 2], F32, tag="den")
                nc.vector.tensor_reduce(
                    out=den[:, :, 0], in_=au_sb[:, slot, :, :],
                    axis=mybir.AxisListType.X, op=mybir.AluOpType.add,
                )
                nc.vector.reciprocal(out=den[:, :, 1], in_=den[:, :, 0])
                # normalize on GPSIMD; quad 3's upper pair feeds P's
                # critical path, so it goes to the faster DVE
                for j in range(4):
                    st = st0 + j
                    eng = nc.gpsimd if (q in (0, 3) and j < 2) else nc.vector
                    eng.tensor_scalar_mul(
                        A_sk[:, st, :], au_sb[:, slot, j, 0:K],
                        den[:, j, 1:2],
                    )

            def emit_r(q):
                st0 = 4 * q
                for j in range(4):
                    st = st0 + j
                    ssl = slice(st * P, (st + 1) * P)
                    ps_r = pR.tile([P, F], F32, tag="r")
                    for h in range(4):
                        nc.tensor.matmul(
                            ps_r[:], xt_sb[:, 2 * h:2 * h + 2, ssl],
                            q_sb[:, 2 * h:2 * h + 2, :],
                            start=(h == 0), stop=(h == 3), perf_mode=DR,
                        )
                    rs = j if q == 0 else 4 + j
                    if j % 2 == 0:
                        nc.scalar.mul(r_sb[:, rs, :], ps_r[:], RSCALE)
                    else:
                        nc.vector.tensor_scalar_mul(
                            r_sb[:, rs, :], ps_r[:], RSCALE)

            def emit_trs(q):
                st0 = 4 * q
                # transposed copies for the final out matmul (fp8 PE
                # transpose needs an output element step of 2)
                ps_t = ptr.tile([K, 4, 256, 2], FP8, tag="tr")
                for j in range(4):
                    nc.tensor.transpose(
                        ps_t[:, j, 0:P, 0], A_sk[:, st0 + j, :], ident8[:]
                    )
                if q != 3:
                    eng = nc.scalar.copy if q == 1 else nc.vector.tensor_copy
                    eng(out=a_ks[:, st0:st0 + 4, :], in_=ps_t[:, :, 0:P, 0])
                else:
                    tr_tiles[q] = ps_t     # evicted post-loop on idle DVE

            for q in range(3):
                ps_a = emit_xc(q)
                emit_norm(q, q, ps_a)
                if q == 0:
                    emit_r(0)
                if q == 1:
                    # P contributions of s-pairs 0-1, off the tail
                    for i in range(2):
                        nc.tensor.matmul(
                            ps_P[:], A_sk[:, 2 * i:2 * i + 2, :],
                            r_sb[:, 2 * i:2 * i + 2, :],
                            start=(i == 0), stop=False,
                            perf_mode=DR, skip_group_check=True,
                        )
                emit_trs(q)

            # quad 3, ordered so every tile-granular RAW edge is true:
            # xc+norm first (exp hits an idle ACT), R next (evictions on
            # ACT/DVE while the norm chain runs), then the M waves (their
            # A_sk readers now sit after only already-done writers), the
            # M eviction + both P closes, and the transposes last
            ps_a = emit_xc(3)
            emit_norm(3, 3, ps_a)
            emit_r(3)
            for i in range(4):
                for dt in range(DT):
                    nc.tensor.matmul(
                        ps_M[:, dt, :],
                        xn_sb[:, 2 * i:2 * i + 2, dt * P:(dt + 1) * P],
                        A_sk[:, 4 + 2 * i:6 + 2 * i, :],
                        start=(i == 0 and dt == 0),
                        stop=(i == 3 and dt == DT - 1),
                        perf_mode=DR, skip_group_check=True,
                    )
            # P contributions of s-pairs 6-7 (R-routed)
            for i in range(2):
                nc.tensor.matmul(
                    ps_P[:], A_sk[:, 12 + 2 * i:14 + 2 * i, :],
                    r_sb[:, 4 + 2 * i:6 + 2 * i, :],
                    start=False, stop=False,
                    perf_mode=DR, skip_group_check=True,
                )
            # mt = (M/4).T so the fp8 scale matches the R-route (x16)
            nc.scalar.mul(mt_sb[:], ps_M[:], 0.25)
            for h in range(4):
                nc.tensor.matmul(
                    ps_P[:], mt_sb[:, 2 * h:2 * h + 2, :],
                    q_sb[:, 2 * h:2 * h + 2, :],
                    start=False, stop=(h == 3), perf_mode=DR,
                    skip_group_check=True,
                )
            emit_trs(3)

            # a_ks for quad 3 on DVE while ACT runs the P chain
            nc.vector.tensor_copy(
                out=a_ks[:, 12:16, :], in_=tr_tiles[3][:, :, 0:P, 0])
        nc.scalar.mul(p_sb[:], ps_P[:], PEVSCALE)

        # ---- out = aff @ P, stores stream per s-tile pair ----------
        with (
            tc.tile_pool(name="pout", bufs=5, space="PSUM") as pout,
            tc.tile_pool(name="osb", bufs=8) as osb,
        ):
            out_v = out_d.ap().rearrange("(t p) f -> p t f", p=P)
            for u in range(ST // 2):
                o_sb = osb.tile([P, 2, F], BF, tag="o_sb")
                for j in range(2):
                    st = 2 * u + j
                    ps_o = pout.tile([P, F], F32, tag="po")
                    nc.tensor.matmul(
                        ps_o[:], a_ks[:, st, :], p_sb[:],
                        start=True, stop=True,
                    )
                    if st % 2 == 0:
                        nc.scalar.mul(o_sb[:, j, :], ps_o[:], OSCALE)
                    else:
                        nc.vector.tensor_scalar_mul(o_sb[:, j, :], ps_o[:], OSCALE)
                if u == 0:
                    # two single-tile stores: the first ships while the
                    # second eviction still runs
                    for j in range(2):
                        nc.sync.dma_start(
                            out=out_v[:, j:j + 1, :], in_=o_sb[:, j:j + 1, :]
                        )
                else:
                    nc.sync.dma_start(
                        out=out_v[:, 2 * u:2 * u + 2, :], in_=o_sb[:]
                    )


def _host_prep(x, splat_centers, splat_log_scales, w_value, w_out):
    """Fold scales/weights; build per-core input maps."""
    x = np.asarray(x, dtype=np.float32)
    centers = np.asarray(splat_centers, dtype=np.float32)
    log_scales = np.asarray(splat_log_scales, dtype=np.float32)
    w_value = np.asarray(w_value, dtype=np.float32)
    w_out = np.asarray(w_out, dtype=np.float32)

    scales = np.clip(np.exp(log_scales), 0.1, 2.0)
    g = (1.0 / (scales * scales)).astype(np.float32)            # [K]
    ctsg = (centers.T * g[None, :]).astype(FP8_NP)              # [D,K]
    c_sq = (centers * centers).sum(axis=1)
    brow = (-0.5 * c_sq * g).astype(np.float32)                 # [K]
    arow = (-0.5 * g).astype(np.float32)                        # [K]
    xsq = (x * x).sum(axis=-1)                                  # [B,S]
    q_full = (w_value.T @ w_out.T) * QSCALE                     # [D,D]

    in_maps = []
    for c in range(8):
        b, j = divmod(c, 2)
        xb = x[b]
        aux = np.zeros((2, K + S), dtype=np.float32)
        aux[0, :K] = brow
        aux[0, K:] = 1.0
        aux[1, :K] = arow
        aux[1, K:] = xsq[b]
        in_maps.append({
            "xt": xb.T.astype(FP8_NP).copy(),
            "xn": xb[512:1536].astype(FP8_NP),
            "q8": q_full[:, j * F:(j + 1) * F].astype(FP8_NP).copy(),
            "cts": ctsg,
            "aux": aux.astype(BF_NP),
        })
    return in_maps


def run_on_hw(in_maps, trace=False, phase="full"):
    key = f"nc_{phase}"
    if key not in _CACHE:
        _CACHE[key] = build_nc(phase)
    return run_bass_kernel_spmd(_CACHE[key], in_maps, list(range(8)), trace=trace)


def kernel(**inputs) -> np.ndarray:
    in_maps = _host_prep(**inputs)
    res = run_on_hw(in_maps)
    out = np.empty((B, S, D), dtype=np.float32)
    for c in range(8):
        b, j = divmod(c, 2)
        out[b][:, j * F:(j + 1) * F] = np.asarray(
            res.results[c]["out"]).astype(np.float32)
    return out
